# revision 26
# baseline (speedup 1.0000x reference)
"""Trainium2 Bass kernel for BasicMoE — v5.

Reference computation (N=8192 tokens, D=1024 in, O=1024 out, E=8 experts):
    gates = softmax(x @ Wg + bg)                        # [N, E]
    out   = sum_e gates[:, e] * (x @ We[e] + be[e])     # [N, O]

Data-parallel over tokens: each core gets 1024 tokens + replicated weights;
no collectives.  The per-core kernel is a single near-zero-bubble PE stream
of 512 bf16 matmuls (K=128, N=1024) at the warm issue floor (~429ns each).

Key design points (each from a measured trace):
  - All start-latency-critical DMAs ride the sync HWDGE ring (first bytes
    ~0.7us after the ~8us engine preamble; the scalar ring lags ~3.5us) in
    exact consumption order: [bg|Wg|xt chunk0], We[0] as 4 k-pair chunks,
    xt chunks 1-3, broadcast-bias, We[1..7] (1 contiguous 2MB DMA each,
    16KB-row descriptors), output stores.  xt chunks 4-7 take the scalar
    ring.  Loading the 2MB bias up-front would steal ~40% of the early HBM
    budget from the critical path (measured), hence its late slot.
  - Matmuls accumulate f32 into a two-bank [128,1024] psum tile; the DVE
    drains both halves with ONE scalar_tensor_tensor per (e,t), and acc /
    bias / output are bf16 (the all-bf16 bias add gets the 2x-packed DVE
    mode; the output upcasts on host).  Host-simulated rel_err of a fully
    bf16 accumulate chain is 6.6e-3 vs the 2e-2 gate; this (f32 psum)
    variant is strictly more accurate.
  - gating(t) is interleaved with expert-0(t) in emission order so the PE
    fills the early DMA-pacing gaps with gating work; softmax skips
    max-subtraction (logits are O(1)) and exp's accum_out gives the row
    sums for free.
"""

import numpy as np
import ml_dtypes

N_TOKENS = 8192
D = 1024   # in dim
O = 1024   # out dim
E = 8      # experts
NCORES = 8
NLOC = N_TOKENS // NCORES   # 1024 tokens per core
KT = D // 128               # 8 k-chunks
TT = NLOC // 128            # 8 token chunks
HDR = 8 + KT * E            # bg(8) + Wg(64) columns prepended to chunk 0

BF16 = ml_dtypes.bfloat16

_CACHE = {}


def _build():
    """Build + compile the per-core Bass graph (same graph on all 8 cores)."""
    import concourse.bass as bass
    import concourse.mybir as mybir
    import concourse.tile as tile
    from concourse import bacc

    dt = mybir.dt
    f32 = dt.float32
    bf16 = dt.bfloat16
    Alu = mybir.AluOpType

    nc = bacc.Bacc(
        "TRN2",
        target_bir_lowering=False,
        debug=False,
        enable_asserts=False,
        num_devices=NCORES,
    )

    # xt: [bg(8) | Wg(64) | chunk0(1024) | chunk1..7(1024 each)], 2KB+ rows.
    xt_d = nc.dram_tensor(
        "xt", [128, HDR + TT * KT * 128], bf16, kind="ExternalInput"
    ).ap()
    # We: [e, p, k*1024+o] — each expert is [128, KT*O], 16KB rows.
    we_d = nc.dram_tensor("Wep", [E, 128, KT * O], bf16, kind="ExternalInput").ap()
    # be broadcast across partitions on host: [128, e*O+o].
    bebc_d = nc.dram_tensor("bebc", [128, E * O], bf16, kind="ExternalInput").ap()
    out_d = nc.dram_tensor("out", [NLOC, O], bf16, kind="ExternalOutput").ap()

    with tile.TileContext(nc) as tc:
        with (
            tc.tile_pool(name="const", bufs=1) as cpool,
            tc.tile_pool(name="xp", bufs=1) as xpool,
            tc.tile_pool(name="wp", bufs=4) as wpool,
            tc.tile_pool(name="ap", bufs=1) as apool,
            tc.tile_pool(name="gp", bufs=1) as gpool,
        ):
            ones = cpool.tile([1, 128], bf16)
            nc.gpsimd.memset(ones[:], 1.0)

            # --- sync-ring queue, in consumption order ------------------
            xc0 = xpool.tile([128, HDR + KT * 128], bf16, name="xc0")
            nc.sync.dma_start(xc0[:], xt_d[:, 0 : HDR + KT * 128])

            we_tiles = [
                wpool.tile([128, KT * O], bf16, tag="we", name=f"we{e}")
                for e in range(E)
            ]
            xts = [xc0]
            for t in range(1, TT):
                xc = xpool.tile([128, KT * 128], bf16, tag="xt", bufs=TT - 1,
                                name=f"xt{t}")
                xts.append(xc)

            def xt_dma(eng, t):
                eng.dma_start(
                    xts[t][:],
                    xt_d[:, HDR + t * KT * 128 : HDR + (t + 1) * KT * 128],
                )

            def we0_dma(kk):
                # k-pair chunk of We[0] (4KB-row descriptors) so the first
                # expert matmuls start ~2us after xt chunk 0 lands.
                nc.sync.dma_start(
                    we_tiles[0][:, kk * O : (kk + 2) * O],
                    we_d[0][:, kk * O : (kk + 2) * O],
                )

            # Sync ring, consumption order: the first We[0] k-pairs lead
            # (they gate the first expert matmul); xt chunks 1-3 slot in
            # where gating consumes them; chunks 4-7 ride the late-starting
            # scalar ring.
            we0_dma(0)
            we0_dma(2)
            xt_dma(nc.sync, 1)
            we0_dma(4)
            we0_dma(6)
            xt_dma(nc.sync, 2)
            xt_dma(nc.sync, 3)
            for t in range(4, TT):
                xt_dma(nc.scalar, t)
            # Broadcast bias after the xt chunks: first needed by the DVE
            # at ~15us, lands ~14us; up-front it would halve the early HBM
            # bandwidth of the latency-critical transfers above.
            bebc = cpool.tile([128, E * O], bf16)
            for be_ in range(E):
                nc.sync.dma_start(
                    bebc[:, be_ * O : (be_ + 1) * O],
                    bebc_d[:, be_ * O : (be_ + 1) * O],
                )
            for e in range(1, E):
                nc.sync.dma_start(we_tiles[e][:], we_d[e])

            acc = apool.tile([128, TT * O], bf16)
            g_f32 = gpool.tile([128, TT * E], f32)
            ssum = gpool.tile([128, TT], f32)
            rec = gpool.tile([128, TT], f32)

            bg_ap = xc0[0:1, 0:E]

            def wg_sl(k):
                return xc0[:, 8 + k * E : 8 + (k + 1) * E]

            def xt_tile(k, t):
                if t == 0:
                    return xc0[:, HDR + k * 128 : HDR + (k + 1) * 128]
                return xts[t][:, k * 128 : (k + 1) * 128]

            def gcol(t, e):
                return g_f32[:, t * E + e : t * E + e + 1]

            def acc_sl(t):
                return acc[:, t * O : (t + 1) * O]

            def be_sl(e):
                return bebc[:, e * O : (e + 1) * O]

            with (
                tc.tile_pool(name="psA", bufs=2, space="PSUM") as psA,
                tc.tile_pool(name="psB", bufs=3, space="PSUM") as psB,
            ):
                def gating(t):
                    zg = psA.tile([128, E], f32, tag="zg", name="zg")
                    for k in range(KT):
                        nc.tensor.matmul(
                            zg[:], xt_tile(k, t), wg_sl(k),
                            start=(k == 0), stop=False,
                        )
                    # + bg (rank-1: ones[1,128].T @ bg[1,E])
                    nc.tensor.matmul(zg[:], ones[:], bg_ap, start=False, stop=True)
                    gs = g_f32[:, t * E : (t + 1) * E]
                    # No max-subtraction: logits are O(1) here, exp is safe.
                    nc.scalar.activation(
                        gs, zg[:], mybir.ActivationFunctionType.Exp,
                        accum_out=ssum[:, t : t + 1],
                    )
                def gating_norm(t):
                    # DVE half of the softmax, emitted AFTER the expert
                    # drain of the same slot: ahead of it in the FIFO it
                    # waits on ACT's exp and blocks the psum drain.
                    gs = g_f32[:, t * E : (t + 1) * E]
                    nc.vector.reciprocal(rec[:, t : t + 1], ssum[:, t : t + 1])
                    nc.vector.tensor_scalar_mul(gs, gs, rec[:, t : t + 1])

                def expert_t(e, t):
                    """k-loop matmuls + gated accumulate + bias for (e, t).

                    The psum tile spans two banks; each matmul targets one
                    512-col bank slice (matmul out must be f32 and fit one
                    bank), but the DVE drains both halves in one op.
                    """
                    last = e == E - 1
                    ps = psB.tile([128, O], f32, tag="mm", name="mm")
                    if e > 0:
                        # Bias add first: it only depends on the previous
                        # expert's acc, so the DVE runs it while the PE is
                        # still streaming this expert's matmuls.
                        nc.vector.scalar_tensor_tensor(
                            acc_sl(t), be_sl(e), gcol(t, e), acc_sl(t),
                            op0=Alu.mult, op1=Alu.add,
                        )
                    for k in range(KT):
                        lhs = xt_tile(k, t)
                        for j in range(2):
                            nc.tensor.matmul(
                                ps[:, j * 512 : (j + 1) * 512],
                                lhs,
                                we_tiles[e][:, k * O + j * 512 : k * O + (j + 1) * 512],
                                start=(k == 0),
                                stop=(k == KT - 1),
                            )
                    if e == 0:
                        nc.vector.tensor_scalar_mul(acc_sl(t), ps[:], gcol(t, 0))
                        nc.vector.scalar_tensor_tensor(
                            acc_sl(t), be_sl(0), gcol(t, 0), acc_sl(t),
                            op0=Alu.mult, op1=Alu.add,
                        )
                    elif last:
                        # Split the final drain + store per 512-col half so
                        # the j=0 store overlaps the j=1 accumulate — the
                        # j=0 psum half is complete one matmul early.
                        for j in range(2):
                            a_j = acc[:, t * O + j * 512 : t * O + (j + 1) * 512]
                            nc.vector.scalar_tensor_tensor(
                                a_j, ps[:, j * 512 : (j + 1) * 512],
                                gcol(t, e), a_j,
                                op0=Alu.mult, op1=Alu.add,
                            )
                            nc.sync.dma_start(
                                out_d[t * 128 : (t + 1) * 128,
                                      j * 512 : (j + 1) * 512],
                                a_j,
                            )
                    else:
                        nc.vector.scalar_tensor_tensor(
                            acc_sl(t), ps[:], gcol(t, e), acc_sl(t),
                            op0=Alu.mult, op1=Alu.add,
                        )

                # Interleave gating with expert 0, one gating group AHEAD:
                # each gating group's first matmul carries a scheduler wait
                # on the completion count of all previously-emitted matmuls,
                # so emitting gating(t+1) before expert0(t) (16 matmuls
                # earlier) retires that wait while expert0(t) streams —
                # measured ~600ns/iteration otherwise.
                gating(0)
                gating_norm(0)
                expert_t(0, 0)
                gating(1)
                gating_norm(1)
                for t in range(1, TT):
                    if t + 1 < TT:
                        gating(t + 1)
                    expert_t(0, t)
                    if t + 1 < TT:
                        gating_norm(t + 1)
                for e in range(1, E):
                    for t in range(TT):
                        expert_t(e, t)

    nc.compile()
    return nc


def _get_nc():
    if "nc" not in _CACHE:
        _CACHE["nc"] = _build()
    return _CACHE["nc"]


def _pack_inputs(x, We, be, Wg, bg):
    """Host-side packing: shard + pre-transpose + bf16 cast.

    Every packed layout is chosen so the device DMA descriptors are
    contiguous >=2KB rows.
    """
    x = np.asarray(x, dtype=np.float32)
    We = np.asarray(We, dtype=np.float32)
    be = np.asarray(be, dtype=np.float32)
    Wg = np.asarray(Wg, dtype=np.float32)
    bg = np.asarray(bg, dtype=np.float32)

    # we_p[e, p, k*O+o] = We[e][k*128+p, o]
    we_p = np.ascontiguousarray(
        We.reshape(E, KT, 128, O).transpose(0, 2, 1, 3).reshape(E, 128, KT * O)
    ).astype(BF16)
    bebc = np.ascontiguousarray(
        np.tile(be.reshape(1, E * O), (128, 1))
    ).astype(BF16)
    # header: bg broadcast (8 cols) | wg[p, k*E+e] (64 cols)
    bg_bc = np.tile(bg.reshape(1, E), (128, 1)).astype(np.float32)
    wg_p = Wg.reshape(KT, 128, E).transpose(1, 0, 2).reshape(128, KT * E)

    in_maps = []
    for i in range(NCORES):
        xs = x[i * NLOC : (i + 1) * NLOC]          # [NLOC, D]
        # xt[p, t*KT*128 + k*128 + n] = xs[t*128+n, k*128+p]
        xt = xs.reshape(TT, 128, KT, 128).transpose(3, 0, 2, 1).reshape(
            128, TT * KT * 128
        )
        xt_full = np.ascontiguousarray(
            np.concatenate([bg_bc, wg_p, xt], axis=1)
        ).astype(BF16)
        in_maps.append({"xt": xt_full, "Wep": we_p, "bebc": bebc})
    return in_maps


def _run(inputs, trace=False):
    """Returns (y_full, BassKernelResults)."""
    from concourse.bass_utils import run_bass_kernel_spmd

    nc = _get_nc()
    in_maps = _pack_inputs(**inputs)
    res = run_bass_kernel_spmd(
        nc, in_maps, core_ids=list(range(NCORES)), trace=trace
    )
    y = np.concatenate(
        [res.results[i]["out"] for i in range(NCORES)], axis=0
    ).astype(np.float32)
    return y, res


def kernel(**inputs):
    y, _ = _run(inputs, trace=False)
    return y
